# revision 1
# baseline (speedup 1.0000x reference)
"""CRF loss (nn_CRFLoss) on 8 Trainium2 NeuronCores.

Strategy
--------
The reference computes, per proposition (B*V = 256 of them):
  logZ via a 128-step forward algorithm over T=66 tags, plus a gold path
  score, then nll = sum(logZ - gold) / 256.

The forward recurrence  alpha' = logsumexp_i(alpha_i + trans_ij) + emit_j
is run entirely in exp space:  with E = exp(trans), F_t = exp(emit_t - k),
  u_{t+1} = (E^T u_t) * F_{t+1}        (one matmul + one elementwise mul)
  logZ    = log(sum_j u_last[j] * exp(end_j)) + k*(S-1)
A fixed pre-scale k ~= log(T) + 1/2 keeps u in a tiny dynamic range
(empirically exp([-10, +6]) for N(0,1) emissions), so no per-step
normalization is needed.

The serial scan latency is halved by splitting it into a FORWARD chain
(alpha, steps 1..64) and a BACKWARD chain (beta, steps 127..65) that meet
in the middle:  Z = sum_j alpha_64[j] * beta_64[j].  Both chains have the
same matmul+multiply step shape (backward uses E instead of E^T as the PE
stationary) and interleave on the Tensor/Vector engines, so the ~64-step
chain latency — not the 127 matmuls — bounds the wall clock.

Matmuls run in fp16 (1 cycle/row on the PE vs 4 for fp32) with fp32 PSUM
accumulation; overall nll error vs the f32 reference is ~1e-6 relative.

Sharding: data-parallel over props — 32 props per core on 8 cores; the
tiny [66,66] transition matrices are replicated. Host side does the cheap
gathers (predicate rows from `score`, gold path score), the exp()
pre-scaling, and the final log+reduction of the per-prop partials.
"""

import os
import sys

import numpy as np

for _p in ("/opt/trn_rl_repo",):
    if os.path.isdir(_p) and _p not in sys.path:
        sys.path.insert(0, _p)

import concourse.bass as bass
import concourse.mybir as mybir
import concourse.tile as tile
from concourse import bacc
from concourse.bass_utils import run_bass_kernel_spmd

B, S, V, T = 32, 128, 8, 66
N_CORES = 8
BV = B * V
P = BV // N_CORES          # 32 props per core
NSTEP = S - 1              # 127 transition steps total
MID = 64                   # forward chain covers steps 1..MID
NBWD_MM = NSTEP - MID      # 63 backward matmuls (steps 127..65)
NF_DEV = NSTEP - 1         # F blocks shipped to device (t=1..126)
KAPPA = float(np.float32(4.7))   # per-step pre-scale, added back at the end

# knobs (test.py may override before first kernel() call)
PROFILE = False
TRACE_TMPDIR = None
F_CHUNK_STEPS = 16         # emissions DMA chunking (steps per chunk)
LAST_RESULTS = None        # BassKernelResults of the last run (for profiling)

_nc_cache = {}


def _build_bass():
    # Bacc (not plain Bass): its finalize() runs move_matmul_waits_to_ldweights
    # + generate_event_semaphores, which split multi-semaphore waits that the
    # TRN2 ISA can't encode on a single instruction.
    nc = bacc.Bacc()
    f32 = mybir.dt.float32
    f16 = mybir.dt.float16

    # E, E^T and both chains' initial states packed into one fp16 tensor ->
    # one DMA -> one semaphore, since PE Matmult only supports a single
    # sync wait.
    NCONST = 2 * T + 2 * P
    c_in = nc.dram_tensor("consts", [T, NCONST], f16, kind="ExternalInput")
    f_in = nc.dram_tensor("f_exp", [T, NF_DEV * P], f16, kind="ExternalInput")
    prod_out = nc.dram_tensor("prod_out", [T, P], f32, kind="ExternalOutput")

    with tile.TileContext(nc) as tc:
        with tc.tile_pool(name="const", bufs=1) as const, \
             tc.tile_pool(name="state", bufs=4) as state, \
             tc.tile_pool(name="ps", bufs=3, space="PSUM") as ps:
            c_sb = const.tile([T, NCONST], f16)
            nc.sync.dma_start(out=c_sb, in_=c_in[:, :])
            E_sb = c_sb[:, 0:T]
            Et_sb = c_sb[:, T:2 * T]
            u0_sb = c_sb[:, 2 * T:2 * T + P]
            w0_sb = c_sb[:, 2 * T + P:2 * T + 2 * P]

            F_sb = const.tile([T, NF_DEV * P], f16)
            # issue chunks from both ends alternately (the forward chain
            # consumes F from t=1 up, the backward chain from t=126 down),
            # with small head chunks so both chains can start ASAP.
            def _ranges(lo, hi, first_small):
                out, c = [], lo
                sizes = [first_small] if first_small else []
                while c < hi:
                    sz = sizes.pop(0) if sizes else F_CHUNK_STEPS
                    out.append((c, min(hi, c + sz)))
                    c = min(hi, c + sz)
                return out
            fwd_chunks = _ranges(0, MID, 4)
            bwd_chunks = [(NF_DEV - b, NF_DEV - a)
                          for (a, b) in _ranges(0, NF_DEV - MID, 4)]
            order, i = [], 0
            while i < max(len(fwd_chunks), len(bwd_chunks)):
                if i < len(fwd_chunks):
                    order.append(fwd_chunks[i])
                if i < len(bwd_chunks):
                    order.append(bwd_chunks[i])
                i += 1
            for c0, c1 in order:
                nc.sync.dma_start(
                    out=F_sb[:, c0 * P:c1 * P], in_=f_in[:, c0 * P:c1 * P]
                )

            u_cur, w_cur = u0_sb, w0_sb
            v_last = None
            for k in range(MID):
                # forward step t = k+1:  u' = (E^T u) * F_{k+1}
                v_ps = ps.tile([T, P], f32, tag="v")
                nc.tensor.matmul(v_ps, E_sb, u_cur, start=True, stop=True)
                v_last = v_ps
                if k < MID - 1:
                    u_nxt = state.tile([T, P], f16, tag="u")
                    nc.vector.tensor_mul(u_nxt, v_ps, F_sb[:, k * P:(k + 1) * P])
                    u_cur = u_nxt
                # backward step (k-th matmul: t = 127-k):  b = E w,
                # then w' = b * F_{126-k}.  At k=62 this applies F_64 — the
                # last forward step's emission — moved onto the backward
                # chain so the forward critical path ends at its matmul:
                # sum_j (v*F)*beta == sum_j v*(F*beta).
                if k < NBWD_MM:
                    b_ps = ps.tile([T, P], f32, tag="b")
                    nc.tensor.matmul(b_ps, Et_sb, w_cur, start=True, stop=True)
                    w_nxt = state.tile([T, P], f16, tag="w")
                    nc.vector.tensor_mul(
                        w_nxt, b_ps, F_sb[:, (125 - k) * P:(126 - k) * P])
                    w_cur = w_nxt

            # meet in the middle: Z_p = sum_j v_64[j,p] * (F*beta)_64[j,p];
            # the column sum + log runs on the host.
            prod_sb = state.tile([T, P], f32, tag="prod")
            nc.vector.tensor_mul(prod_sb, v_last, w_cur)
            nc.sync.dma_start(out=prod_out[:, :], in_=prod_sb)

    nc.finalize()
    return nc


def _get_nc():
    key = ("crf-fb", T, P, NSTEP, MID, F_CHUNK_STEPS)
    if key not in _nc_cache:
        _nc_cache[key] = _build_bass()
    return _nc_cache[key]


def kernel(score, transitions, start_transitions, end_transitions,
           v_label, role_label):
    global LAST_RESULTS
    score = np.asarray(score, dtype=np.float32)
    transitions = np.asarray(transitions, dtype=np.float32)
    start_transitions = np.asarray(start_transitions, dtype=np.float32)
    end_transitions = np.asarray(end_transitions, dtype=np.float32)
    vl = np.asarray(v_label).astype(np.int64)
    rl = np.asarray(role_label).astype(np.int64)

    # gather predicate rows: emissions[b*V+v] = score[b, v_label[b,v]]  [BV,S,T]
    em = np.take_along_axis(score, vl[:, :, None, None], axis=1).reshape(BV, S, T)
    tags = rl.reshape(BV, S)

    # gold path score (host, f64)
    ar = np.arange(BV)
    emit_sc = em[ar[:, None], np.arange(S)[None, :], tags].astype(np.float64).sum(-1)
    tr64 = transitions.astype(np.float64)
    trans_sc = tr64[tags[:, :-1], tags[:, 1:]].sum(-1)
    gold = (start_transitions.astype(np.float64)[tags[:, 0]] + emit_sc
            + trans_sc + end_transitions.astype(np.float64)[tags[:, -1]])

    # device inputs
    E = np.exp(transitions)                                   # [T,T] f32
    u0 = np.exp(start_transitions[:, None] + em[:, 0, :].T)   # [T,BV] f32
    # F[j, t, p] = exp(em[p, t+1, j] - kappa); exp(end) folded into the last
    # step, which seeds the backward chain (w_init = F_127 * 1).
    Ft = np.exp(np.transpose(em[:, 1:, :], (2, 1, 0)) - np.float32(KAPPA))
    Ft[:, -1, :] *= np.exp(end_transitions)[:, None]

    nc = _get_nc()
    in_maps = []
    E16 = E.astype(np.float16)
    Et16 = np.ascontiguousarray(E.T).astype(np.float16)
    for m in range(N_CORES):
        sl = slice(m * P, (m + 1) * P)
        consts = np.concatenate(
            [E16, Et16, u0[:, sl].astype(np.float16),
             Ft[:, -1, sl].astype(np.float16)], axis=1)
        in_maps.append({
            "consts": np.ascontiguousarray(consts),
            "f_exp": np.ascontiguousarray(
                Ft[:, :NF_DEV, sl].astype(np.float16)).reshape(T, NF_DEV * P),
        })

    kwargs = {}
    if PROFILE:
        kwargs.update(trace=True, tmpdir=TRACE_TMPDIR)
    res = run_bass_kernel_spmd(nc, in_maps, list(range(N_CORES)), **kwargs)
    LAST_RESULTS = res

    prod = np.concatenate(
        [res.results[m]["prod_out"] for m in range(N_CORES)], axis=1)  # [T, BV]
    logz = np.log(prod.astype(np.float64).sum(0)) + KAPPA * NSTEP
    nll = (logz - gold).sum() / BV
    return np.float32(nll)



# revision 2
# speedup vs baseline: 1.0817x; 1.0817x over previous
"""CRF loss (nn_CRFLoss) on 8 Trainium2 NeuronCores — raw-Bass version.

Same math as kernel.py (exp-space forward/backward meet-in-the-middle scan,
fp16 matmuls, fixed kappa pre-scale), but the hot loop is hand-scheduled
raw Bass instead of Tile:

  - exactly one semaphore wait per instruction, attached inline (no
    standalone EVENT_SEMAPHORE churn on the Vector queue),
  - PSUM/SBUF double buffers whose reuse safety is implied by queue order
    (no recycle waits at all),
  - input DMAs issued on otherwise-idle queues (consts on Sync, the four
    emission chunks on Scalar) so the PE/DVE queues only carry real work,
  - optional PE warm-up matmuls + in-loop dummy matmuls to coax the HAM
    clock gate to 2.4 GHz (knobs below).

Per-queue programs (P = 32 props/core, T = 66 tags):
  PE:   [warmup]  LDW+MMf0, LDW+MMb0, LDW+MMf1, ... LDW+MMf63
  DVE:  TTf0, TTb0, TTf1, TTb1, ... TTb62, TTprod
  Sync: consts DMA, output DMA (pre-queued, waits on TTprod)
  Scal: f_exp chunk DMAs (fwd head, bwd head, fwd tail, bwd tail)
"""

import os
import sys

import numpy as np

for _p in ("/opt/trn_rl_repo",):
    if os.path.isdir(_p) and _p not in sys.path:
        sys.path.insert(0, _p)

import concourse.bass as bass
import concourse.mybir as mybir
from concourse import bacc
from concourse.bass_utils import run_bass_kernel_spmd

B, S, V, T = 32, 128, 8, 66
N_CORES = 8
BV = B * V
P = BV // N_CORES          # 32 props per core
NSTEP = S - 1              # 127 transition steps total
MID = 64                   # fwd chain: MMf k=0..63 (steps 1..64)
NBWD = NSTEP - MID         # 63 bwd matmuls (steps 127..65)
NF_DEV = NSTEP - 1         # F blocks on device (t=1..126)
KAPPA = float(np.float32(4.7))

# knobs (test.py may override before first kernel() call)
PROFILE = False
TRACE_TMPDIR = None
LAST_RESULTS = None
NWARM = 0                  # pre-scan warmup matmuls
WARM_N = 256               # their moving free dim
DUM = 0                    # dummy matmuls after each real matmul
DUM_N = 64                 # their moving free dim
FHEAD = 4                  # F blocks per chain shipped inside the consts DMA

_nc_cache = {}

f16 = mybir.dt.float16
f32 = mybir.dt.float32


def _build_bass():
    nc = bacc.Bacc()
    NCONST = 2 * T + 2 * P + 2 * FHEAD * P

    c_in = nc.dram_tensor("consts", [T, NCONST], f16, kind="ExternalInput")
    f_in = nc.dram_tensor("f_exp", [T, NF_DEV * P], f16, kind="ExternalInput")
    prod_out = nc.dram_tensor("prod_out", [T, P], f32, kind="ExternalOutput")

    c_sb = nc.alloc_sbuf_tensor("c_sb", [T, NCONST], f16)
    f_sb = nc.alloc_sbuf_tensor("f_sb", [T, NF_DEV * P], f16)
    u_sb = [nc.alloc_sbuf_tensor(f"u_sb{i}", [T, P], f16) for i in range(2)]
    w_sb = [nc.alloc_sbuf_tensor(f"w_sb{i}", [T, P], f16) for i in range(2)]
    prod_sb = nc.alloc_sbuf_tensor("prod_sb", [T, P], f32)
    warm_sb = nc.alloc_sbuf_tensor("warm_sb", [128, 256 + 32], f16)

    vps = [nc.alloc_psum_tensor(f"vps{i}", [T, P], f32) for i in range(2)]
    bps = [nc.alloc_psum_tensor(f"bps{i}", [T, P], f32) for i in range(2)]
    warm_ps = nc.alloc_psum_tensor("warm_ps", [32, 512], f32)

    E_sb = c_sb[:, 0:T]
    Et_sb = c_sb[:, T:2 * T]
    u0_sb = c_sb[:, 2 * T:2 * T + P]
    w0_sb = c_sb[:, 2 * T + P:2 * T + 2 * P]
    fh_base = 2 * T + 2 * P            # fwd blocks 0..FHEAD-1
    bh_base = fh_base + FHEAD * P      # bwd blocks NF_DEV-FHEAD..NF_DEV-1

    def f_fwd(k):
        # emission block k as consumed by the fwd chain
        if k < FHEAD:
            return c_sb[:, fh_base + k * P:fh_base + (k + 1) * P]
        return f_sb[:, k * P:(k + 1) * P]

    def f_bwd(k):
        # emission block NF_DEV-1-k as consumed by the bwd chain
        blk = NF_DEV - 1 - k
        if k < FHEAD:
            pos = blk - (NF_DEV - FHEAD)
            return c_sb[:, bh_base + pos * P:bh_base + (pos + 1) * P]
        return f_sb[:, blk * P:(blk + 1) * P]

    s_c = nc.alloc_semaphore("s_c")
    s_ff0 = nc.alloc_semaphore("s_ff0")
    s_ff1 = nc.alloc_semaphore("s_ff1")
    s_fb0 = nc.alloc_semaphore("s_fb0")
    s_fb1 = nc.alloc_semaphore("s_fb1")
    s_mf = nc.alloc_semaphore("s_mf")
    s_tf = nc.alloc_semaphore("s_tf")
    s_mb = nc.alloc_semaphore("s_mb")
    s_tb = nc.alloc_semaphore("s_tb")
    s_pr = nc.alloc_semaphore("s_pr")
    s_out = nc.alloc_semaphore("s_out")

    # ---- DMA issue, spread over idle queues so configs run in parallel:
    # Sync: consts, bwd head, output.  Vector: fwd head (config time is free
    # there — DVE has nothing to do until the first multiply anyway).
    # Scalar: the two tails.
    FF0_BLKS = 16            # fwd blocks 0..15 / 16..62
    FB0_BLKS = 16            # bwd blocks 110..125 / 63..109

    def fdma(eng, a, b, sem):
        eng.dma_start(
            out=f_sb[:, a * P:b * P], in_=f_in[:, a * P:b * P]
        ).then_inc(sem, 16)

    nc.sync.dma_start(out=c_sb[:, :], in_=c_in[:, :]).then_inc(s_c, 16)
    fdma(nc.scalar, FHEAD, FF0_BLKS, s_ff0)
    fdma(nc.gpsimd, NF_DEV - FB0_BLKS, NF_DEV - FHEAD, s_fb0)
    fdma(nc.scalar, FF0_BLKS, MID - 1, s_ff1)
    fdma(nc.scalar, MID - 1, NF_DEV - FB0_BLKS, s_fb1)
    # output DMA pre-queued; Sync sits on this wait for the rest of the run
    nc.sync.dma_start(out=prod_out[:, :], in_=prod_sb[:, :]).wait_op(
        s_pr, 1, "sem-ge").then_inc(s_out, 16)

    # ---- PE warmup (no consumers; garbage data is fine) ----
    warm_w = warm_sb[:, 256:256 + 32]
    for _ in range(NWARM):
        nc.tensor.matmul(warm_ps[:, 0:WARM_N], warm_w, warm_sb[:, 0:WARM_N],
                         start=True, stop=True, skip_group_check=True)

    def dummy_mms():
        for _ in range(DUM):
            nc.tensor.matmul(warm_ps[:, 0:DUM_N], warm_w,
                             warm_sb[:, 0:DUM_N],
                             start=True, stop=True, skip_group_check=True)

    # hold LDW(E) of the first matmul until consts have landed
    nc.tensor.wait_ge(s_c, 16)

    def pe_step(out_ps, wts, mv):
        # explicit LDW + non-self-loading MM: the MM keeps its sem wait, the
        # LDW stays wait-free so the PE reorder window pulls it ahead of the
        # in-flight previous matmul (self-loading MMs get their wait moved
        # onto the LDW by walrus, putting the 130ns weight load on the
        # critical path).
        nc.tensor.ldweights(wts)
        mm = nc.tensor.matmul(out_ps, wts, mv, start=True, stop=True)
        mm.ins.ldweights = False
        return mm

    # ---- the scan ----
    for k in range(MID):
        # fwd matmul, step t=k+1: v = E^T u_k
        mv = u0_sb if k == 0 else u_sb[(k - 1) % 2][:, :]
        mm = pe_step(vps[k % 2][:, :], E_sb, mv)
        if k > 0:
            mm.wait_op(s_tf, k, "sem-ge")
        mm.then_inc(s_mf)
        dummy_mms()

        # bwd matmul, step t=127-k: b = E w_k
        if k < NBWD:
            mv = w0_sb if k == 0 else w_sb[(k - 1) % 2][:, :]
            mm = pe_step(bps[k % 2][:, :], Et_sb, mv)
            if k > 0:
                mm.wait_op(s_tb, k, "sem-ge")
            mm.then_inc(s_mb)
            dummy_mms()

        # fwd multiply: u_{k+1} = v * F_k   (k = 0..62)
        if k < MID - 1:
            if k == FHEAD:
                nc.vector.wait_ge(s_ff0, 16)
            elif k == FF0_BLKS:
                nc.vector.wait_ge(s_ff1, 16)
            tt = nc.vector.tensor_mul(
                u_sb[k % 2][:, :], vps[k % 2][:, :], f_fwd(k))
            tt.wait_op(s_mf, k + 1, "sem-ge")
            tt.then_inc(s_tf)

        # bwd multiply: w_{k+1} = b * F_{125-k}   (k = 0..62)
        if k < NBWD:
            if k == FHEAD:
                nc.vector.wait_ge(s_fb0, 16)
            elif k == FB0_BLKS:
                nc.vector.wait_ge(s_fb1, 16)
            tt = nc.vector.tensor_mul(
                w_sb[k % 2][:, :], bps[k % 2][:, :], f_bwd(k))
            tt.wait_op(s_mb, k + 1, "sem-ge")
            tt.then_inc(s_tb)

    # meet in the middle: prod = v_63 * w_final  (w_final = TTb#62 -> w_sb[0])
    tt = nc.vector.tensor_mul(prod_sb[:, :], vps[(MID - 1) % 2][:, :],
                              w_sb[(NBWD - 1) % 2][:, :])
    tt.wait_op(s_mf, MID, "sem-ge")
    tt.then_inc(s_pr)

    nc.finalize()
    return nc


def _get_nc():
    key = ("crf-raw", T, P, NSTEP, MID, NWARM, WARM_N, DUM, DUM_N, FHEAD)
    if key not in _nc_cache:
        _nc_cache[key] = _build_bass()
    return _nc_cache[key]


def kernel(score, transitions, start_transitions, end_transitions,
           v_label, role_label):
    global LAST_RESULTS
    score = np.asarray(score, dtype=np.float32)
    transitions = np.asarray(transitions, dtype=np.float32)
    start_transitions = np.asarray(start_transitions, dtype=np.float32)
    end_transitions = np.asarray(end_transitions, dtype=np.float32)
    vl = np.asarray(v_label).astype(np.int64)
    rl = np.asarray(role_label).astype(np.int64)

    em = np.take_along_axis(score, vl[:, :, None, None], axis=1).reshape(BV, S, T)
    tags = rl.reshape(BV, S)

    # gold path score (host, f64)
    ar = np.arange(BV)
    emit_sc = em[ar[:, None], np.arange(S)[None, :], tags].astype(np.float64).sum(-1)
    tr64 = transitions.astype(np.float64)
    trans_sc = tr64[tags[:, :-1], tags[:, 1:]].sum(-1)
    gold = (start_transitions.astype(np.float64)[tags[:, 0]] + emit_sc
            + trans_sc + end_transitions.astype(np.float64)[tags[:, -1]])

    # device inputs
    E = np.exp(transitions)
    u0 = np.exp(start_transitions[:, None] + em[:, 0, :].T)
    Ft = np.exp(np.transpose(em[:, 1:, :], (2, 1, 0)) - np.float32(KAPPA))
    Ft[:, -1, :] *= np.exp(end_transitions)[:, None]

    nc = _get_nc()
    in_maps = []
    E16 = E.astype(np.float16)
    Et16 = np.ascontiguousarray(E.T).astype(np.float16)
    for m in range(N_CORES):
        sl = slice(m * P, (m + 1) * P)
        F16 = Ft[:, :NF_DEV, sl].astype(np.float16)
        consts = np.concatenate(
            [E16, Et16, u0[:, sl].astype(np.float16),
             Ft[:, -1, sl].astype(np.float16),
             F16[:, :FHEAD].reshape(T, FHEAD * P),
             F16[:, NF_DEV - FHEAD:].reshape(T, FHEAD * P)], axis=1)
        in_maps.append({
            "consts": np.ascontiguousarray(consts),
            "f_exp": np.ascontiguousarray(F16).reshape(T, NF_DEV * P),
        })

    kwargs = {}
    if PROFILE:
        kwargs.update(trace=True, tmpdir=TRACE_TMPDIR)
    res = run_bass_kernel_spmd(nc, in_maps, list(range(N_CORES)), **kwargs)
    LAST_RESULTS = res

    prod = np.concatenate(
        [res.results[m]["prod_out"] for m in range(N_CORES)], axis=1)
    logz = np.log(prod.astype(np.float64).sum(0)) + KAPPA * NSTEP
    nll = (logz - gold).sum() / BV
    return np.float32(nll)
